# revision 1
# baseline (speedup 1.0000x reference)
"""BankedLinear (MoE-style banked linear) Trainium2 kernel.

Reference computation (per token t, with k=2 selected banks):
    out[t] = sum_k prob[t,k] * (x[t] @ W[sel[t,k]] + bias[sel[t,k]])

Strategy (expert-parallel over 8 NeuronCores):
  - Core c owns banks [8c, 8c+8).  Its weight slab (8 x 512 x 512 = 8 MB of
    fp32 information) is the dominant, unavoidable HBM traffic; each bank is
    read exactly once system-wide.
  - Host routes token-bank pairs to cores by selected bank, pre-scales each
    gathered token row by its probability, transposes to [in_feature, slot],
    and pads to CAP=32 slots per bank.
  - Precision/speed: fp32 matmul runs at 1/4 rate on the PE and bf16 at full
    rate, so both x and W are split hi/lo into two bf16 halves on the host
    (same total bytes as fp32) and each bank's product is computed as
    xh@wh + xh@wl + xl@wh accumulated in fp32 PSUM (~1e-6 rel error).
  - All arrays are pre-swizzled on the host into SBUF layout so every DMA is
    a single large contiguous 2D transfer.
  - Bias is folded in on the host (one gather + multiply-add over 1024
    pairs); host scatter-adds the per-pair device results into the output.

Fixed shapes: B=2, T=256, K=2, IN=OUT=512, NB=64 banks, 8 cores.
Capacity: 32 slots/bank (binomial mean 16, sd ~4; overflow pairs — none for
realistic routing — are handled exactly on the host as a fallback).
"""

import numpy as np
from contextlib import ExitStack

B, T, KSEL = 2, 256, 2
IN, OUT, NB = 512, 512, 64
NCORES = 8
BPC = NB // NCORES          # banks per core = 8
CAP = 32                    # padded token slots per bank
SLOTS = BPC * CAP           # 256 dispatch rows per core
PCHUNK = 128                # contraction chunk (SBUF partition dim)
KC = IN // PCHUNK           # 4 contraction chunks
GROUPS = SLOTS // 128       # output row groups of 128

_cache = {}


def _build_nc():
    """Build the Bass/Tile program (one SPMD NeuronCore program)."""
    import concourse.tile as tile
    import concourse.mybir as mybir
    from concourse import bacc

    f32 = mybir.dt.float32
    bf16 = mybir.dt.bfloat16
    nc = bacc.Bacc("TRN2", target_bir_lowering=False, debug=False,
                   num_devices=NCORES)
    # host-pre-swizzled SBUF layouts: partition dim first, contiguous free dim
    xth = nc.dram_tensor("xth", [PCHUNK, KC * SLOTS], bf16,
                         kind="ExternalInput").ap()
    xtl = nc.dram_tensor("xtl", [PCHUNK, KC * SLOTS], bf16,
                         kind="ExternalInput").ap()
    wh = nc.dram_tensor("wh", [BPC, PCHUNK, KC * OUT], bf16,
                        kind="ExternalInput").ap()
    wl = nc.dram_tensor("wl", [BPC, PCHUNK, KC * OUT], bf16,
                        kind="ExternalInput").ap()
    y = nc.dram_tensor("y", [SLOTS, OUT], f32, kind="ExternalOutput").ap()

    from concourse.tile import add_dep_helper

    def chain(dep_chain, binst, reason):
        # pin scheduler order: binst depends on the previous link
        if dep_chain:
            add_dep_helper(binst.ins, dep_chain[-1].ins, sync=False,
                           reason=reason)
        dep_chain.append(binst)

    KH = 2                      # kc chunks per weight DMA (256KB granularity)
    with tile.TileContext(nc) as tc:
        with ExitStack() as ctx:
            xpool = ctx.enter_context(tc.tile_pool(name="xp", bufs=2))
            wpool = ctx.enter_context(
                tc.tile_pool(name="wp", bufs=2 * BPC * KC // KH))
            ypool = ctx.enter_context(tc.tile_pool(name="yp", bufs=GROUPS))
            pspool = ctx.enter_context(
                tc.tile_pool(name="ps", bufs=3, space="PSUM"))

            # token dispatch first on the sync ring: every matmul needs it,
            # so it must land before the weight stream floods HBM
            xh_sb = xpool.tile([PCHUNK, KC * SLOTS], bf16, tag="xh")
            xl_sb = xpool.tile([PCHUNK, KC * SLOTS], bf16, tag="xl")


            ysbs = []
            for g in range(GROUPS):
                ysb_g = ypool.tile([128, OUT], f32, tag="y")
                ysbs.append(ysb_g)

            wq = []    # sync-ring DMA chain (keeps FIFO = compute order)
            mq = []    # PE matmul chain (keeps bank order = arrival order)
            chain(wq, nc.sync.dma_start(xh_sb[:], xth[:]), "xt first")
            chain(wq, nc.sync.dma_start(xl_sb[:], xtl[:]), "xt first")

            # Banks processed in pairs. The even bank computes in PE column
            # group 0, the odd bank in column group 1 (tile_position), so
            # their matmuls overlap in the array. Each bank accumulates in
            # its OWN psum bank (separate tiles) so the per-bank start=True
            # has_written clear cannot disturb its neighbour.
            for p in range(BPC // 2):
                whs, wls = [[], []], [[], []]
                for q in range(2):
                    j = 2 * p + q
                    for kh in range(KC // KH):
                        ks = slice(kh * KH * OUT, (kh + 1) * KH * OUT)
                        wh_t = wpool.tile([PCHUNK, KH * OUT], bf16, tag="w")
                        chain(wq, nc.sync.dma_start(wh_t[:], wh[j, :, ks]),
                              "weight ring order")
                        whs[q].append(wh_t)
                        wl_t = wpool.tile([PCHUNK, KH * OUT], bf16, tag="w")
                        chain(wq, nc.sync.dma_start(wl_t[:], wl[j, :, ks]),
                              "weight ring order")
                        wls[q].append(wl_t)

                psA = pspool.tile([CAP, OUT], f32, tag="psA")
                psB = pspool.tile([2 * CAP, OUT], f32, tag="psB")
                outs = (psA[:], psB[CAP:2 * CAP, :])
                nmm = 3 * KC
                i = 0
                first_mm = None
                for kc in range(KC):
                    kh, ko = divmod(kc, KH)
                    ws = slice(ko * OUT, (ko + 1) * OUT)
                    for term in range(3):
                        a_sb = xh_sb if term < 2 else xl_sb
                        for q in range(2):
                            j = 2 * p + q
                            xs = slice(kc * SLOTS + j * CAP,
                                       kc * SLOTS + (j + 1) * CAP)
                            b_t = whs[q][kh] if term != 1 else wls[q][kh]
                            mm = nc.tensor.matmul(
                                outs[q], a_sb[:, xs], b_t[:, ws],
                                start=(i < 2), stop=(i >= 2 * nmm - 2),
                                tile_position=(0, q * CAP),
                                skip_group_check=True)
                            if first_mm is None:
                                first_mm = mm
                                chain(mq, mm, "pair compute order")
                            i += 1
                g, gq = divmod(p, 2)
                nc.vector.tensor_copy(
                    ysbs[g][gq * 2 * CAP:gq * 2 * CAP + CAP, :], psA[:])
                nc.vector.tensor_copy(
                    ysbs[g][gq * 2 * CAP + CAP:(gq + 1) * 2 * CAP, :],
                    psB[CAP:2 * CAP, :])

            # output stores on the sync ring after all weights (ring is free
            # by then; HWDGE has lower first-byte latency than SWDGE), at
            # 2-bank granularity so the last store only waits on the last
            # two banks' copies
            for h in range(2 * GROUPS):
                g, hq = divmod(h, 2)
                chain(wq, nc.sync.dma_start(
                    y[h * 64:(h + 1) * 64, :],
                    ysbs[g][hq * 64:(hq + 1) * 64, :]), "y after weights")
    nc.compile()
    return nc


def _get_nc():
    if "nc" not in _cache:
        _cache["nc"] = _build_nc()
    return _cache["nc"]


def _split_hilo(a32):
    """fp32 array -> (hi, lo) bf16 halves with a32 ~= hi + lo."""
    import ml_dtypes
    bf = ml_dtypes.bfloat16
    hi = a32.astype(bf)
    lo = (a32 - hi.astype(np.float32)).astype(bf)
    return hi, lo


def _swizzle_x(xt):
    """[IN, SLOTS] -> [128, KC*SLOTS] with free index (kc, slot)."""
    return np.ascontiguousarray(
        xt.reshape(KC, PCHUNK, SLOTS).transpose(1, 0, 2).reshape(
            PCHUNK, KC * SLOTS))


def _swizzle_w(w):
    """[BPC, IN, OUT] -> [BPC, 128, KC*OUT] with free index (kc, out)."""
    return np.ascontiguousarray(
        w.reshape(BPC, KC, PCHUNK, OUT).transpose(0, 2, 1, 3).reshape(
            BPC, PCHUNK, KC * OUT))


def _route(X, sel, prob):
    """Group token-bank pairs by bank, build per-core dispatch arrays.

    Returns (in_maps, slot_tok [NCORES,SLOTS] int64 (-1=pad), overflow list
    of (token, bank, prob))."""
    NT = X.shape[0]
    pair_tok = np.repeat(np.arange(NT, dtype=np.int64), KSEL)
    pair_bank = sel.reshape(-1)
    pair_p = prob.reshape(-1)

    order = np.argsort(pair_bank, kind="stable")
    counts = np.bincount(pair_bank, minlength=NB)
    starts = np.concatenate(([0], np.cumsum(counts)))

    slot_tok = np.full((NCORES, SLOTS), -1, dtype=np.int64)
    slot_p = np.zeros((NCORES, SLOTS), dtype=np.float32)
    overflow = []
    for b in range(NB):
        c, j = divmod(b, BPC)
        s0, s1 = starts[b], starts[b + 1]
        take = min(s1 - s0, CAP)
        idx = order[s0:s0 + take]
        slot_tok[c, j * CAP: j * CAP + take] = pair_tok[idx]
        slot_p[c, j * CAP: j * CAP + take] = pair_p[idx]
        for i in order[s0 + take:s1]:
            overflow.append((int(pair_tok[i]), b, float(pair_p[i])))
    return slot_tok, slot_p, overflow


def _combine(ys, slot_tok, X, sel, prob, weights, bias, overflow):
    NT = X.shape[0]
    out = np.zeros((NT, OUT), dtype=np.float32)
    for c in range(NCORES):
        tok = slot_tok[c]
        valid = tok >= 0
        np.add.at(out, tok[valid], ys[c][valid])
    # bias term for every pair (device computes x @ W only)
    for k in range(KSEL):
        out += prob[:, k, None] * bias[sel[:, k]]
    # exact host fallback for capacity-overflow pairs (expected: none)
    for t, b, p in overflow:
        out[t] += p * (X[t] @ weights[b])
    return out


def _run_device(in_maps, trace=False, **kwargs):
    from concourse.bass_utils import run_bass_kernel_spmd
    return run_bass_kernel_spmd(_get_nc(), in_maps,
                                core_ids=list(range(NCORES)),
                                trace=trace, **kwargs)


def kernel(_trace=False, _bass_results=None, **inputs):
    tensor = np.asarray(inputs["tensor"], dtype=np.float32)
    sel = np.asarray(inputs["bank_selections"]).astype(np.int64)
    prob = np.asarray(inputs["bank_probabilities"], dtype=np.float32)
    weights = np.asarray(inputs["weights"], dtype=np.float32)
    bias = np.asarray(inputs["bias"], dtype=np.float32)

    NT = tensor.shape[0] * tensor.shape[1]
    X = tensor.reshape(NT, IN)
    sel2 = sel.reshape(NT, KSEL)
    prob2 = prob.reshape(NT, KSEL)

    slot_tok, slot_p, overflow = _route(X, sel2, prob2)

    in_maps = []
    for c in range(NCORES):
        tok = slot_tok[c]
        rows = X[np.where(tok >= 0, tok, 0)] * slot_p[c][:, None]
        xt = np.ascontiguousarray(rows.T)              # [IN, SLOTS] fp32
        xh, xl = _split_hilo(xt)
        w32 = weights[c * BPC:(c + 1) * BPC]           # (8, 512, 512) fp32
        wwh, wwl = _split_hilo(w32)
        in_maps.append({
            "xth": _swizzle_x(xh), "xtl": _swizzle_x(xl),
            "wh": _swizzle_w(wwh), "wl": _swizzle_w(wwl),
        })

    res = _run_device(in_maps, trace=_trace)
    if _bass_results is not None:
        _bass_results.append(res)
    ys = [res.results[c]["y"] for c in range(NCORES)]

    out = _combine(ys, slot_tok, X, sel2, prob2, weights, bias, overflow)
    return out.reshape(tensor.shape[0], tensor.shape[1], OUT)



# revision 5
# speedup vs baseline: 1.6353x; 1.6353x over previous
"""BankedLinear (MoE-style banked linear) Trainium2 kernel.

Reference computation (per token t, with k=2 selected banks):
    out[t] = sum_k prob[t,k] * (x[t] @ W[sel[t,k]] + bias[sel[t,k]])

Strategy (expert-parallel over 8 NeuronCores):
  - Core c owns banks [8c, 8c+8).  Its weight slab is the dominant HBM
    traffic; each bank is read exactly once system-wide.
  - Host routes token-bank pairs to cores by selected bank, pre-scales each
    gathered token row by its probability, transposes to [in_feature, slot],
    and pads to CAP=32 slots per bank.
  - Precision: everything runs in a single bf16 matmul term (x_bf16 @ W_bf16
    accumulated in fp32 PSUM, ~2e-3 rel error, well under the 2e-2 gate).
    This halves weight DMA bytes and cuts PE work 3x vs an fp32-faithful
    hi/lo split.
  - Weights stream as a few ~1MB HWDGE DMAs on the sync ring (big transfers
    amortize the ~600ns per-DMA issue cost and use 8KB/partition descriptor
    lines); the last pairs are split by contraction chunk so the final
    matmuls overlap the stream tail.  x and y ride the scalar (Activation)
    HWDGE ring so they never queue behind the weight stream.
  - PE: banks run 2 pairs per PSUM bank with 4-way column tiling
    (tile_position col groups 0-3), so up to 4 banks' matmuls stream
    concurrently through the 128x128 array.  A single start=True matmul
    clears each PSUM bank; later matmuls overwrite-where-clear /
    accumulate-where-set (per-element has_written semantics).
  - Bias is folded in on the host (one gather + multiply-add over 1024
    pairs); host scatter-adds the per-pair device results into the output.

Fixed shapes: B=2, T=256, K=2, IN=OUT=512, NB=64 banks, 8 cores.
Capacity: 32 slots/bank (binomial mean 16, sd ~4; overflow pairs -- none for
realistic routing -- are handled exactly on the host as a fallback).
"""

import numpy as np
from contextlib import ExitStack

B, T, KSEL = 2, 256, 2
IN, OUT, NB = 512, 512, 64
NCORES = 8
BPC = NB // NCORES          # banks per core = 8
CAP = 32                    # padded token slots per bank
SLOTS = BPC * CAP           # 256 dispatch rows per core
PCHUNK = 128                # contraction chunk (SBUF partition dim)
KC = IN // PCHUNK           # 4 contraction chunks
NPAIR = BPC // 2            # 4 bank pairs
GROUPS = 2                  # PSUM banks / output row groups of 128

# weight DMA chunking per pair (kc ranges); later pairs split finer so the
# final matmuls overlap the end of the weight stream
WCHUNKS = {
    0: [(0, 4)],
    1: [(0, 4)],
    2: [(0, 2), (2, 4)],
    3: [(0, 1), (1, 2), (2, 3), (3, 4)],
}

_cache = {}


def _build_nc():
    """Build the Bass/Tile program (one SPMD NeuronCore program)."""
    import concourse.tile as tile
    import concourse.mybir as mybir
    from concourse import bacc
    from concourse.tile import add_dep_helper

    f32 = mybir.dt.float32
    bf16 = mybir.dt.bfloat16
    nc = bacc.Bacc("TRN2", target_bir_lowering=False, debug=False,
                   num_devices=NCORES)
    # host-pre-swizzled layouts: partition dim first, contiguous free dim
    # xt free index: (kc, slot)
    xt = nc.dram_tensor("xt", [PCHUNK, KC * SLOTS], bf16,
                        kind="ExternalInput").ap()
    # w free index: (pair, kc, q, out) so both pair-sized and kc-sized DMA
    # slices are per-partition contiguous
    w = nc.dram_tensor("w", [PCHUNK, BPC * KC * OUT], bf16,
                       kind="ExternalInput").ap()
    # y free index: (group, out); row = slot within group
    y = nc.dram_tensor("y", [PCHUNK, GROUPS * OUT], f32,
                       kind="ExternalOutput").ap()

    def chain(dep_chain, binst, reason):
        # pin scheduler order: binst depends on the previous link
        if dep_chain:
            add_dep_helper(binst.ins, dep_chain[-1].ins, sync=False,
                           reason=reason)
        dep_chain.append(binst)

    with tile.TileContext(nc) as tc:
        with ExitStack() as ctx:
            xpool = ctx.enter_context(tc.tile_pool(name="xp", bufs=1))
            wpool = ctx.enter_context(tc.tile_pool(name="wp", bufs=1))
            ypool = ctx.enter_context(tc.tile_pool(name="yp", bufs=GROUPS))
            pspool = ctx.enter_context(
                tc.tile_pool(name="ps", bufs=GROUPS, space="PSUM"))

            xt_sb = xpool.tile([PCHUNK, KC * SLOTS], bf16, tag="x")
            ysbs = [ypool.tile([PCHUNK, OUT], f32, tag="y", name=f"ysb{g}")
                    for g in range(GROUPS)]
            pss = [pspool.tile([PCHUNK, OUT], f32, tag="ps", name=f"ps{g}")
                   for g in range(GROUPS)]

            sq = []   # scalar HWDGE ring: xt load, then y stores
            wq = []   # sync HWDGE ring: weight stream in compute order
            mq = []   # PE chain: per-pair compute order

            chain(sq, nc.scalar.dma_start(xt_sb[:], xt[:]), "xt first")

            wtiles = {}
            for p in range(NPAIR):
                wtiles[p] = []
                for (k0, k1) in WCHUNKS[p]:
                    t = wpool.tile([PCHUNK, (k1 - k0) * 2 * OUT], bf16,
                                   name=f"w{p}_{k0}")
                    src = w[:, (p * KC + k0) * 2 * OUT:
                            (p * KC + k1) * 2 * OUT]
                    chain(wq, nc.sync.dma_start(t[:], src), "w ring order")
                    wtiles[p].append((k0, k1, t))

            for p in range(NPAIR):
                g, h = divmod(p, 2)
                first = None
                for kc in range(KC):
                    for (k0, k1, t) in wtiles[p]:
                        if k0 <= kc < k1:
                            break
                    for q in range(2):
                        j = 2 * p + q
                        c = 2 * h + q       # PE column group 0..3
                        rhs = t[:, ((kc - k0) * 2 + q) * OUT:
                                ((kc - k0) * 2 + q + 1) * OUT]
                        lhsT = xt_sb[:, kc * SLOTS + j * CAP:
                                     kc * SLOTS + (j + 1) * CAP]
                        outap = pss[g][32 * c:32 * (c + 1), :]
                        mm = nc.tensor.matmul(
                            outap, lhsT, rhs,
                            # first mm touching this PSUM bank clears it;
                            # everyone else overwrites-where-clear
                            start=(h == 0 and kc == 0 and q == 0),
                            stop=(h == 1 and kc == KC - 1 and q == 1),
                            tile_position=(0, 32 * c),
                            skip_group_check=True)
                        if first is None:
                            first = mm
                            chain(mq, mm, "pair compute order")
                if h == 1:
                    nc.vector.tensor_copy(ysbs[g][:], pss[g][:])
                    chain(sq, nc.scalar.dma_start(
                        y[:, g * OUT:(g + 1) * OUT], ysbs[g][:]),
                        "y after copy")
    nc.compile()
    return nc


def _get_nc():
    if "nc" not in _cache:
        _cache["nc"] = _build_nc()
    return _cache["nc"]


def _bf16(a32):
    import ml_dtypes
    return a32.astype(ml_dtypes.bfloat16)


def _swizzle_x(xt):
    """[IN, SLOTS] -> [128, KC*SLOTS] with free index (kc, slot)."""
    return np.ascontiguousarray(
        xt.reshape(KC, PCHUNK, SLOTS).transpose(1, 0, 2).reshape(
            PCHUNK, KC * SLOTS))


def _swizzle_w(wb):
    """[BPC, IN, OUT] bf16 -> [128, NPAIR*KC*2*OUT], free (pair, kc, q, out)."""
    # (pair, q, kc, row, out) -> (row, pair, kc, q, out)
    return np.ascontiguousarray(
        wb.reshape(NPAIR, 2, KC, PCHUNK, OUT).transpose(3, 0, 2, 1, 4)
        .reshape(PCHUNK, NPAIR * KC * 2 * OUT))


def _route(X, sel, prob):
    """Group token-bank pairs by bank, build per-core dispatch arrays.

    Returns (slot_tok [NCORES,SLOTS] int64 (-1=pad), slot_p, overflow list
    of (token, bank, prob))."""
    NT = X.shape[0]
    pair_tok = np.repeat(np.arange(NT, dtype=np.int64), KSEL)
    pair_bank = sel.reshape(-1)
    pair_p = prob.reshape(-1)

    order = np.argsort(pair_bank, kind="stable")
    counts = np.bincount(pair_bank, minlength=NB)
    starts = np.concatenate(([0], np.cumsum(counts)))

    slot_tok = np.full((NCORES, SLOTS), -1, dtype=np.int64)
    slot_p = np.zeros((NCORES, SLOTS), dtype=np.float32)
    overflow = []
    for b in range(NB):
        c, j = divmod(b, BPC)
        s0, s1 = starts[b], starts[b + 1]
        take = min(s1 - s0, CAP)
        idx = order[s0:s0 + take]
        slot_tok[c, j * CAP: j * CAP + take] = pair_tok[idx]
        slot_p[c, j * CAP: j * CAP + take] = pair_p[idx]
        for i in order[s0 + take:s1]:
            overflow.append((int(pair_tok[i]), b, float(pair_p[i])))
    return slot_tok, slot_p, overflow


def _combine(ys, slot_tok, X, sel, prob, weights, bias, overflow):
    NT = X.shape[0]
    out = np.zeros((NT, OUT), dtype=np.float32)
    for c in range(NCORES):
        tok = slot_tok[c]
        valid = tok >= 0
        np.add.at(out, tok[valid], ys[c][valid])
    # bias term for every pair (device computes x @ W only)
    for k in range(KSEL):
        out += prob[:, k, None] * bias[sel[:, k]]
    # exact host fallback for capacity-overflow pairs (expected: none)
    for t, b, p in overflow:
        out[t] += p * (X[t] @ weights[b])
    return out


def _run_device(in_maps, trace=False, **kwargs):
    from concourse.bass_utils import run_bass_kernel_spmd
    return run_bass_kernel_spmd(_get_nc(), in_maps,
                                core_ids=list(range(NCORES)),
                                trace=trace, **kwargs)


def kernel(_trace=False, _bass_results=None, **inputs):
    tensor = np.asarray(inputs["tensor"], dtype=np.float32)
    sel = np.asarray(inputs["bank_selections"]).astype(np.int64)
    prob = np.asarray(inputs["bank_probabilities"], dtype=np.float32)
    weights = np.asarray(inputs["weights"], dtype=np.float32)
    bias = np.asarray(inputs["bias"], dtype=np.float32)

    NT = tensor.shape[0] * tensor.shape[1]
    X = tensor.reshape(NT, IN)
    sel2 = sel.reshape(NT, KSEL)
    prob2 = prob.reshape(NT, KSEL)

    slot_tok, slot_p, overflow = _route(X, sel2, prob2)

    in_maps = []
    for c in range(NCORES):
        tok = slot_tok[c]
        rows = X[np.where(tok >= 0, tok, 0)] * slot_p[c][:, None]
        xt = np.ascontiguousarray(rows.T)              # [IN, SLOTS] fp32
        wb = _bf16(weights[c * BPC:(c + 1) * BPC])     # (8, 512, 512) bf16
        in_maps.append({
            "xt": _swizzle_x(_bf16(xt)),
            "w": _swizzle_w(wb),
        })

    res = _run_device(in_maps, trace=_trace)
    if _bass_results is not None:
        _bass_results.append(res)
    # y: [128, GROUPS*OUT] f32; row r of group g is slot g*128 + r
    ys = []
    for c in range(NCORES):
        yflat = res.results[c]["y"]
        ys.append(np.concatenate(
            [yflat[:, g * OUT:(g + 1) * OUT] for g in range(GROUPS)],
            axis=0))

    out = _combine(ys, slot_tok, X, sel2, prob2, weights, bias, overflow)
    return out.reshape(tensor.shape[0], tensor.shape[1], OUT)
